# revision 28
# baseline (speedup 1.0000x reference)
"""Trainium2 Bass kernel for nn_BiEvidenceNet.

Model (B=1024, R=512, D=256):
    width  = clip(exp(log_width), 1e-3, 50)                  (R,D)
    t_low  = center - width/2 ; t_high = center + width/2    (R,D)
    kappa  = clip(exp(log_kappa), 0.5, 50)                   scalar
    low    = sigmoid(kappa*(t_low - x))   high = sigmoid(kappa*(x - t_high))
    evidence[b,r] = sum_d m*(el*(2*low-1) + eh*(2*high-1))   m=sig(mask), el/eh=tanh(e_*)
    z = sigmoid(6*(evidence - t));  y = z @ head_w.T + head_b

Key identity: 2*sigmoid(u)-1 = tanh(u/2). When t_low / t_high are constant
across the rule axis (true at init: center == 0, log_width == 0 -- verified at
runtime), the (B,R,D) broadcast collapses to two matmuls:
    T_low[b,d]  = tanh(kappa/2*(tau_low[d]  - x[b,d]))
    T_high[b,d] = tanh(kappa/2*(x[b,d] - tau_high[d]))
    evidence    = T_low @ (m*el).T + T_high @ (m*eh).T
Sharding: data-parallel over B, 128 rows/core; (R,D) params replicated.
On-core layout keeps D on partitions (2 k-tiles of 128) so both matmul
operands are naturally transposed; evidence lands as (128 b, 512 r) in PSUM,
-t enters as two rank-1 matmul updates, and the head is a DVE
multiply+reduce over the free (rule) axis followed by a PE transpose so the
output leaves as one contiguous row (a partition-strided 4B-per-partition
store pays ~7us of HWDGE semaphore latency).

Toolchain constraint baked in throughout: this walrus encodes at most ONE
sync wait per instruction, so the instruction graph is arranged so every op
has a single-semaphore dependency (see the `ones`/`wcheck` covering ops).
"""

import numpy as np

B, R, D = 1024, 512, 256
N_CORES = 8
BS = B // N_CORES          # batch rows per core
KT = D // 128              # contraction k-tiles
BETA = 6.0
TRIM_TAIL = True           # skip Tile's sem-clear + second barrier (one-shot NEFF)

_F32 = np.float32


def _single_wait_tile_context(nc, tile):
    """TileContext whose tail carries at most one sync wait per instruction."""
    from concourse.vector_clock import ScopedClock, VectorClock

    class SingleWaitTileContext(tile.TileContext):
        def _drain_and_barrier(self, tick_clock, wait_clock):
            gc = tick_clock.global_clock
            n = len(gc)
            for proc in range(n):
                if gc[proc] <= 0:
                    continue
                vec = VectorClock([gc[i] if i == proc else 0 for i in range(n)])
                inst = self.nc.sync.nop(nofuse=True)
                wait_clock.add_sem_waits(inst.ins, ScopedClock({None: vec}))
            # the NOP chain above already waited out every proc, so the drain
            # itself needs no waits (walrus would reject a multi-wait drain)
            self.nc.sync.drain()
            self.nc.all_engine_barrier()
            assert self.sems is not None
            popped = self.nc._tile_sem_poison_stack.pop()
            assert popped is self._sem_poison
            if not TRIM_TAIL:
                self.nc.clear_and_free_semaphores(
                    list(self.sems.allocated().values()))
                self.nc.all_engine_barrier()

    return SingleWaitTileContext(nc)


def _build_nc(scale_lo: float, scale_hi: float, head_b: float):
    import concourse.bass as bass
    import concourse.mybir as mybir
    from concourse import tile

    f32 = mybir.dt.float32
    f32r = mybir.dt.float32r
    AF = mybir.ActivationFunctionType
    ALU = mybir.AluOpType

    nc = bass.Bass()
    # xb packs the x shard (transposed) with the two per-partition activation
    # bias columns so each T activation depends on exactly one DMA semaphore.
    d_xb = nc.declare_dram_parameter("xb", [KT, 128, BS + 2], f32, isOutput=False)
    d_maskT = nc.declare_dram_parameter("maskT", [KT, 128, R], f32, isOutput=False)
    d_elT = nc.declare_dram_parameter("elT", [KT, 128, R], f32, isOutput=False)
    d_ehT = nc.declare_dram_parameter("ehT", [KT, 128, R], f32, isOutput=False)
    d_t = nc.declare_dram_parameter("t_row", [1, R], f32, isOutput=False)
    # head_w broadcast to 128 partitions, with a 128x128 identity appended
    # (used to transpose y into a single contiguous output row)
    d_wbi = nc.declare_dram_parameter("wbi", [BS, R + BS], f32, isOutput=False)
    d_y = nc.declare_dram_parameter("y", [1, BS], f32, isOutput=True)

    with _single_wait_tile_context(nc, tile) as tc:
        with (
            tc.tile_pool(name="sb", bufs=1) as sb,
            tc.tile_pool(name="ps", bufs=1, space="PSUM") as ps,
        ):
            mkt = sb.tile([128, KT, R], f32, tag="mkt")
            elt = sb.tile([128, KT, R], f32, tag="elt")
            eht = sb.tile([128, KT, R], f32, tag="eht")
            xt = sb.tile([128, KT, BS + 2], f32, tag="xt")
            tr = sb.tile([1, R], f32, tag="tr")
            wbi = sb.tile([BS, R + BS], f32, tag="wbi")

            # Big replicated params stream on HWDGE queues, one DMA per
            # (tensor, k) so the k0 halves arrive ~2us before the k1 halves
            # and compute can start on them.  Small tensors go through
            # GpSimd-triggered SWDGE so their triggers don't serialize behind
            # the big ones on the sync engine.
            for k in range(KT):
                nc.sync.dma_start(mkt[:, k, :], d_maskT[k])
                nc.sync.dma_start(elt[:, k, :], d_elT[k])
                nc.sync.dma_start(eht[:, k, :], d_ehT[k])
            nc.gpsimd.dma_start(xt[:], d_xb[:].rearrange("k p b -> p k b"))
            nc.gpsimd.dma_start(tr[:], d_t[:])
            nc.gpsimd.dma_start(wbi[:], d_wbi[:])

            # per-(k, side) rule prep + T tiles, so each ACT waits on the one
            # DMA it needs; matmul operand tiles are float32r (PE full rate)
            tlo = sb.tile([128, KT, BS], f32r, tag="tlo")
            thi = sb.tile([128, KT, BS], f32r, tag="thi")
            m = sb.tile([128, KT, R], f32, tag="m")
            el = sb.tile([128, KT, R], f32, tag="el")
            eh = sb.tile([128, KT, R], f32, tag="eh")
            a_t = sb.tile([128, KT, R], f32r, tag="a_t")
            b_t = sb.tile([128, KT, R], f32r, tag="b_t")
            # covering ops: o_xx reads the DVE product it follows, so a PE
            # wait on o_xx's DVE tick transitively covers that product
            o_cov = sb.tile([1, 4, BS], f32r, tag="o_cov")
            negt = sb.tile([1, R], f32r, tag="negt")
            nc.vector.tensor_scalar_mul(negt[:], tr[:], -1.0)
            wcheck = sb.tile([1, 1], f32, tag="wcheck")
            nc.vector.tensor_scalar_mul(wcheck[:], wbi[0:1, 0:1], 1.0)

            prods = []
            for k in range(KT):
                nc.scalar.activation(m[:, k, :], mkt[:, k, :], AF.Sigmoid)
                nc.scalar.activation(el[:, k, :], elt[:, k, :], AF.Tanh)
                nc.scalar.activation(eh[:, k, :], eht[:, k, :], AF.Tanh)
                nc.scalar.activation(
                    tlo[:, k, :], xt[:, k, 0:BS], AF.Tanh,
                    bias=xt[:, k, BS:BS + 1], scale=scale_lo,
                )
                nc.scalar.activation(
                    thi[:, k, :], xt[:, k, 0:BS], AF.Tanh,
                    bias=xt[:, k, BS + 1:BS + 2], scale=scale_hi,
                )
                for prod, lhs, rhs_m in ((a_t, tlo, el), (b_t, thi, eh)):
                    i = len(prods)
                    nc.vector.tensor_mul(prod[:, k, :], m[:, k, :], rhs_m[:, k, :])
                    with tc.high_priority():
                        nc.vector.tensor_scalar(
                            o_cov[0:1, i, :], prod[0:1, k, 0:BS],
                            0.0, 1.0, ALU.mult, ALU.add)
                    prods.append((lhs[:, k, :], prod[:, k, :]))

            ev = ps.tile([128, R], f32, tag="ev")

            # evidence - t in one PSUM bank.  Before each data matmul, a tiny
            # covering matmul carries the DVE wait for that operand
            # (single-wait-per-instruction rule); the first one doubles as
            # the real -t rank-1 update, the rest are N=1 throwaways.
            from concourse.tile_rust import add_dep_helper

            cov_ps = ps.tile([128, 1], f32, tag="cov_ps")
            prev = None
            for i, (lhsT, rhs) in enumerate(prods):
                if i == 0:
                    cov = nc.tensor.matmul(ev[:], o_cov[0:1, 0, :], negt[:],
                                           start=True, stop=False)
                else:
                    # plain fp32 (f32r has a min free-dim ISA restriction)
                    cov = nc.tensor.matmul(cov_ps[:],
                                           o_cov[0:1, i, :].bitcast(f32),
                                           negt[0:1, 0:1].bitcast(f32),
                                           start=True, stop=True)
                data = nc.tensor.matmul(ev[:], lhsT, rhs,
                                        start=False, stop=(i == len(prods) - 1))
                # pin PE order cov_i < data_i < cov_{i+1}: the throwaway cov
                # matmuls have no data deps on the ev chain, and the coverage
                # argument (one sync wait per instruction) relies on order
                if prev is not None:
                    add_dep_helper(cov.ins, prev.ins, sync=False,
                                   reason="single-wait coverage order")
                add_dep_helper(data.ins, cov.ins, sync=False,
                               reason="single-wait coverage order")
                prev = data

            # dummy matmul whose only dependency is the wbi DMA: it makes the
            # PE observe that queue's semaphore so the final transpose matmul
            # (which also reads wbi) needs only its DVE wait
            scratch_ps = ps.tile([128, 1], f32, tag="scratch_ps")
            nc.tensor.matmul(scratch_ps[:], wbi[:, R:R + BS], wbi[:, R:R + 1],
                             start=True, stop=True)

            z = sb.tile([128, R], f32, tag="z")
            nc.scalar.activation(z[:], ev[:], AF.Sigmoid, scale=BETA)

            # head: y[b] = sum_r z*w + head_b, then transpose the (128,1)
            # column into a (1,128) row on the PE so the output DMA is one
            # contiguous packet instead of 4 bytes per partition
            zw = sb.tile([128, R], f32, tag="zw")
            yt = sb.tile([128, 1], f32, tag="yt")
            nc.vector.tensor_mul(zw[:], z[:], wbi[:, 0:R])
            nc.vector.tensor_reduce(
                yt[:], zw[:], axis=mybir.AxisListType.X, op=ALU.add)
            nc.vector.tensor_scalar_add(yt[:], yt[:], head_b)

            yrow_ps = ps.tile([1, BS], f32, tag="yrow_ps")
            nc.tensor.matmul(yrow_ps[:], yt[:], wbi[:, R:R + BS],
                             start=True, stop=True)
            yrow = sb.tile([1, BS], f32, tag="yrow")
            nc.scalar.activation(yrow[:], yrow_ps[:], AF.Identity)
            nc.sync.dma_start(d_y[:], yrow[:])

    nc.finalize()
    return nc


def _fast_path_inputs(x, mask, e_low, e_high, tau_lo, tau_hi, kappa, t, head_w):
    """Build the per-core input maps (host work = transposes/slicing only)."""
    khalf = _F32(kappa) / _F32(2.0)
    blo = (khalf * tau_lo).astype(_F32).reshape(KT, 128)
    bhi = (-khalf * tau_hi).astype(_F32).reshape(KT, 128)
    maskT = np.ascontiguousarray(mask.T.reshape(KT, 128, R), dtype=_F32)
    elT = np.ascontiguousarray(e_low.T.reshape(KT, 128, R), dtype=_F32)
    ehT = np.ascontiguousarray(e_high.T.reshape(KT, 128, R), dtype=_F32)
    t_row = np.ascontiguousarray(t.reshape(1, R), dtype=_F32)
    wbi = np.empty((BS, R + BS), dtype=_F32)
    wbi[:, :R] = head_w.reshape(1, R)
    wbi[:, R:] = np.eye(BS, dtype=_F32)
    xT = np.ascontiguousarray(x.T, dtype=_F32)  # (D, B)

    in_maps = []
    for i in range(N_CORES):
        xb = np.empty((KT, 128, BS + 2), dtype=_F32)
        xb[:, :, :BS] = xT[:, i * BS:(i + 1) * BS].reshape(KT, 128, BS)
        xb[:, :, BS] = blo
        xb[:, :, BS + 1] = bhi
        in_maps.append({
            "xb": xb, "maskT": maskT, "elT": elT, "ehT": ehT,
            "t_row": t_row, "wbi": wbi,
        })
    return in_maps, float(-khalf), float(khalf)


def _reference_numpy(x, center, log_width, e_low, e_high, mask, log_kappa, t,
                     head_w, head_b):
    """General fallback, exact reference semantics in fp32 numpy (chunked)."""
    width = np.clip(np.exp(log_width, dtype=_F32), 1e-3, 50.0).astype(_F32)
    t_low = (center - _F32(0.5) * width).astype(_F32)
    t_high = (center + _F32(0.5) * width).astype(_F32)
    kappa = np.clip(np.exp(_F32(log_kappa)), 0.5, 50.0).astype(_F32)

    def sig(v):
        return _F32(0.5) * (np.tanh(_F32(0.5) * v) + _F32(1.0))

    m = sig(mask.astype(_F32))
    el = np.tanh(e_low.astype(_F32))
    eh = np.tanh(e_high.astype(_F32))
    out = np.empty(x.shape[0], dtype=_F32)
    for s in range(0, x.shape[0], 64):
        xc = x[s:s + 64].astype(_F32)
        low = sig(kappa * (t_low[None] - xc[:, None, :]))
        high = sig(kappa * (xc[:, None, :] - t_high[None]))
        evidence = np.sum(
            m[None] * (el[None] * (2 * low - 1) + eh[None] * (2 * high - 1)),
            axis=2, dtype=_F32)
        z = sig(_F32(BETA) * (evidence - t[None].astype(_F32)))
        out[s:s + 64] = z @ head_w.reshape(-1).astype(_F32) + _F32(head_b)
    return out


def kernel_with_stats(trace=False, **inputs):
    x = np.asarray(inputs["x"], dtype=_F32)
    center = np.asarray(inputs["center"], dtype=_F32)
    log_width = np.asarray(inputs["log_width"], dtype=_F32)
    e_low = np.asarray(inputs["e_low"], dtype=_F32)
    e_high = np.asarray(inputs["e_high"], dtype=_F32)
    mask = np.asarray(inputs["mask"], dtype=_F32)
    log_kappa = np.asarray(inputs["log_kappa"], dtype=_F32)
    t = np.asarray(inputs["t"], dtype=_F32)
    head_w = np.asarray(inputs["head_w"], dtype=_F32)
    head_b = np.asarray(inputs["head_b"], dtype=_F32)

    assert x.shape == (B, D) and mask.shape == (R, D)

    # fast-path structural check: thresholds constant across the rule axis
    width = np.clip(np.exp(log_width), 1e-3, 50.0).astype(_F32)
    t_low = (center - _F32(0.5) * width).astype(_F32)
    t_high = (center + _F32(0.5) * width).astype(_F32)
    if not (np.all(t_low == t_low[0:1]) and np.all(t_high == t_high[0:1])):
        out = _reference_numpy(x, center, log_width, e_low, e_high, mask,
                               log_kappa, t, head_w, head_b)
        return out, None

    from concourse.bass_utils import run_bass_kernel_spmd

    kappa = np.clip(np.exp(_F32(log_kappa)), 0.5, 50.0).astype(_F32)
    in_maps, scale_lo, scale_hi = _fast_path_inputs(
        x, mask, e_low, e_high, t_low[0], t_high[0], kappa, t, head_w)

    nc = _build_nc(scale_lo, scale_hi, float(head_b.reshape(-1)[0]))
    res = run_bass_kernel_spmd(nc, in_maps, list(range(N_CORES)), trace=trace)
    out = np.concatenate(
        [res.results[i]["y"].reshape(BS) for i in range(N_CORES)]).astype(_F32)
    return out, res


def kernel(**inputs):
    out, _ = kernel_with_stats(**inputs)
    return out


# revision 33
# speedup vs baseline: 1.1442x; 1.1442x over previous
"""Trainium2 Bass kernel for nn_BiEvidenceNet.

Model (B=1024, R=512, D=256):
    width  = clip(exp(log_width), 1e-3, 50)                  (R,D)
    t_low  = center - width/2 ; t_high = center + width/2    (R,D)
    kappa  = clip(exp(log_kappa), 0.5, 50)                   scalar
    low    = sigmoid(kappa*(t_low - x))   high = sigmoid(kappa*(x - t_high))
    evidence[b,r] = sum_d m*(el*(2*low-1) + eh*(2*high-1))   m=sig(mask), el/eh=tanh(e_*)
    z = sigmoid(6*(evidence - t));  y = z @ head_w.T + head_b

Key identity: 2*sigmoid(u)-1 = tanh(u/2). When t_low / t_high are constant
across the rule axis (true at init: center == 0, log_width == 0 -- verified at
runtime), the (B,R,D) broadcast collapses to two matmuls:
    T_low[b,d]  = tanh(kappa/2*(tau_low[d]  - x[b,d]))
    T_high[b,d] = tanh(kappa/2*(x[b,d] - tau_high[d]))
    evidence    = T_low @ (m*el).T + T_high @ (m*eh).T

Sharding: 2D, 4 batch shards x 2 rule shards over the 8 cores.  Rule-sharded
partial y vectors (each with head_b/2) are summed on the host during the
gather.  On-core layout keeps D on partitions (2 k-tiles of 128) so both
matmul operands are naturally transposed; evidence accumulates per b-half in
PSUM (b on partitions, rules on free), -t enters as a rank-1 matmul, and the
head is a DVE multiply+reduce over the free (rule) axis followed by a PE
transpose so the output leaves as contiguous rows (a 4B-per-partition store
pays microseconds of HWDGE semaphore latency).

Toolchain constraint baked in throughout: this walrus encodes at most ONE
sync wait per instruction.  Every op is arranged to have a single-semaphore
dependency: cheap ACT "touch" ops observe the DVE products so each PE matmul
needs only its ACT wait, and a dummy matmul pulls the wbi DMA tick onto the
PE for the final transpose.  float32r operands run the PE at ~2x the plain
fp32 rate.
"""

import numpy as np

B, R, D = 1024, 512, 256
N_CORES = 8
NB = 4                      # batch shards
NR = 2                      # rule shards
B2 = B // NB                # batch rows per core (256)
R2 = R // NR                # rules per core (256)
BH = 128                    # b-half (psum partition dim)
KT = D // 128               # contraction k-tiles
BETA = 6.0
TRIM_TAIL = True            # skip Tile's sem-clear + second barrier (one-shot NEFF)

_F32 = np.float32


def _single_wait_tile_context(nc, tile):
    """TileContext whose tail carries at most one sync wait per instruction."""
    from concourse.vector_clock import ScopedClock, VectorClock

    class SingleWaitTileContext(tile.TileContext):
        def _drain_and_barrier(self, tick_clock, wait_clock):
            gc = tick_clock.global_clock
            n = len(gc)
            for proc in range(n):
                if gc[proc] <= 0:
                    continue
                vec = VectorClock([gc[i] if i == proc else 0 for i in range(n)])
                inst = self.nc.sync.nop(nofuse=True)
                wait_clock.add_sem_waits(inst.ins, ScopedClock({None: vec}))
            # the NOP chain above already waited out every proc, so the drain
            # itself needs no waits (walrus would reject a multi-wait drain)
            self.nc.sync.drain()
            self.nc.all_engine_barrier()
            assert self.sems is not None
            popped = self.nc._tile_sem_poison_stack.pop()
            assert popped is self._sem_poison
            if not TRIM_TAIL:
                self.nc.clear_and_free_semaphores(
                    list(self.sems.allocated().values()))
                self.nc.all_engine_barrier()

    return SingleWaitTileContext(nc)


def _build_nc(scale_lo: float, scale_hi: float, head_b_half: float):
    import concourse.bass as bass
    import concourse.mybir as mybir
    from concourse import tile

    f32 = mybir.dt.float32
    f32r = mybir.dt.float32r
    bf16 = mybir.dt.bfloat16
    AF = mybir.ActivationFunctionType
    ALU = mybir.AluOpType

    nc = bass.Bass()
    # xb packs the x shard (transposed) with the two per-partition activation
    # bias columns so each T activation depends on exactly one DMA semaphore
    d_xb = nc.declare_dram_parameter("xb", [KT, 128, B2 + 2], f32, isOutput=False)
    d_maskT = nc.declare_dram_parameter("maskT", [KT, 128, R2], f32, isOutput=False)
    d_elT = nc.declare_dram_parameter("elT", [KT, 128, R2], f32, isOutput=False)
    d_ehT = nc.declare_dram_parameter("ehT", [KT, 128, R2], f32, isOutput=False)
    d_t = nc.declare_dram_parameter("t_row", [1, R2], f32, isOutput=False)
    # head_w shard broadcast to 128 partitions + a 128x128 identity appended
    d_wbi = nc.declare_dram_parameter("wbi", [BH, R2 + BH], f32, isOutput=False)
    d_y = nc.declare_dram_parameter("y", [2, BH], f32, isOutput=True)

    with _single_wait_tile_context(nc, tile) as tc:
        with (
            tc.tile_pool(name="sb", bufs=1) as sb,
            tc.tile_pool(name="ps", bufs=1, space="PSUM") as ps,
        ):
            mkt = sb.tile([128, KT, R2], f32, tag="mkt")
            elt = sb.tile([128, KT, R2], f32, tag="elt")
            eht = sb.tile([128, KT, R2], f32, tag="eht")
            xt = sb.tile([128, KT, B2 + 2], f32, tag="xt")
            tr = sb.tile([1, R2], f32, tag="tr")
            wbi = sb.tile([BH, R2 + BH], f32, tag="wbi")

            # big param shards on HWDGE queues, one DMA per (tensor, k) so k0
            # halves arrive early; small tensors via GpSimd-triggered SWDGE
            # so their triggers don't serialize behind the big ones
            for k in range(KT):
                nc.sync.dma_start(mkt[:, k, :], d_maskT[k])
                nc.sync.dma_start(elt[:, k, :], d_elT[k])
                nc.sync.dma_start(eht[:, k, :], d_ehT[k])
            nc.gpsimd.dma_start(xt[:], d_xb[:].rearrange("k p b -> p k b"))
            nc.gpsimd.dma_start(tr[:], d_t[:])
            nc.gpsimd.dma_start(wbi[:], d_wbi[:])

            tlo = sb.tile([128, KT, B2], f32r, tag="tlo")
            thi = sb.tile([128, KT, B2], f32r, tag="thi")
            m = sb.tile([128, KT, R2], f32, tag="m")
            el = sb.tile([128, KT, R2], f32, tag="el")
            eh = sb.tile([128, KT, R2], f32, tag="eh")
            a_t = sb.tile([128, KT, R2], f32r, tag="a_t")
            b_t = sb.tile([128, KT, R2], f32r, tag="b_t")

            # rank-1 (-t) operands produced on ACT so the rank-1 matmuls
            # carry a single ACT wait
            ones = sb.tile([1, B2], f32r, tag="ones")
            negt = sb.tile([1, R2], f32r, tag="negt")
            nc.scalar.activation(ones[:], xt[0:1, 0, 0:B2], AF.Identity,
                                 bias=1.0, scale=0.0)
            nc.scalar.activation(negt[:], tr[:], AF.Identity, scale=-1.0)

            # DVE touch of wbi so the head's DVE ops need only the ACT wait
            wcheck = sb.tile([1, 1], f32, tag="wcheck")
            nc.vector.tensor_scalar_mul(wcheck[:], wbi[0:1, 0:1], 1.0)

            # per-(k, side) prep
            prods = []
            for k in range(KT):
                nc.scalar.activation(m[:, k, :], mkt[:, k, :], AF.Sigmoid)
                nc.scalar.activation(el[:, k, :], elt[:, k, :], AF.Tanh)
                nc.vector.tensor_mul(a_t[:, k, :], m[:, k, :], el[:, k, :])
                nc.scalar.activation(eh[:, k, :], eht[:, k, :], AF.Tanh)
                nc.vector.tensor_mul(b_t[:, k, :], m[:, k, :], eh[:, k, :])
                nc.scalar.activation(
                    tlo[:, k, :], xt[:, k, 0:B2], AF.Tanh,
                    bias=xt[:, k, B2:B2 + 1], scale=scale_lo,
                )
                nc.scalar.activation(
                    thi[:, k, :], xt[:, k, 0:B2], AF.Tanh,
                    bias=xt[:, k, B2 + 1:B2 + 2], scale=scale_hi,
                )
                for side, prod, lhs in ((0, a_t, tlo), (1, b_t, thi)):
                    prods.append((lhs, prod, k))

            # dummy matmul whose only dependency is the wbi DMA: the PE
            # observes that queue so the final transpose matmul needs only
            # its DVE wait
            scratch_ps = ps.tile([128, 1], f32, tag="scratch_ps")
            nc.tensor.matmul(scratch_ps[:], wbi[:, R2:R2 + BH],
                             wbi[:, R2:R2 + 1], start=True, stop=True)

            # evidence - t per b-half, each in its own PSUM bank.  Before the
            # data matmuls of each (k, side) product, a tiny bf16 covering
            # matmul reads the product so the PE observes its DVE tick; the
            # data matmuls then carry only their ACT wait (single-wait rule).
            # Coverage relies on PE program order, pinned via add_dep_helper.
            from concourse.tile_rust import add_dep_helper

            ev0 = ps.tile([128, R2], f32, tag="ev0")
            ev1 = ps.tile([128, R2], f32, tag="ev1")
            evs = [ev0, ev1]
            cov_ps = ps.tile([1, 1], f32, tag="cov_ps")
            prev = None
            for h in range(2):
                r1 = nc.tensor.matmul(evs[h][:], ones[0:1, h * BH:(h + 1) * BH],
                                      negt[:], start=True, stop=False)
                prev = r1
            for i, (lhs, prod, k) in enumerate(prods):
                last = i == len(prods) - 1
                pb = prod[0:1, k, 0:1].bitcast(bf16)[0:1, 0:1]
                cov = nc.tensor.matmul(cov_ps[:], pb, pb, start=True, stop=True)
                add_dep_helper(cov.ins, prev.ins, sync=False,
                               reason="single-wait coverage order")
                prev = cov
                for h in range(2):
                    data = nc.tensor.matmul(
                        evs[h][:], lhs[:, k, h * BH:(h + 1) * BH],
                        prod[:, k, :], start=False, stop=last)
                    add_dep_helper(data.ins, prev.ins, sync=False,
                                   reason="single-wait coverage order")
                    prev = data

            # z and the head, per b-half; partial y (this core's rule shard)
            z = sb.tile([128, 2, R2], f32, tag="z")
            zw = sb.tile([128, 2, R2], f32, tag="zw")
            yt2 = sb.tile([128, 2], f32, tag="yt2")
            for h in range(2):
                nc.scalar.activation(z[:, h, :], evs[h][:], AF.Sigmoid,
                                     scale=BETA)
                nc.vector.tensor_mul(zw[:, h, :], z[:, h, :], wbi[:, 0:R2])
                nc.vector.tensor_reduce(
                    yt2[:, h:h + 1], zw[:, h, :],
                    axis=mybir.AxisListType.X, op=ALU.add)
            nc.vector.tensor_scalar_add(yt2[:], yt2[:], head_b_half)

            # transpose partial y into contiguous rows: yp[h, n] = yt2[n, h]
            yp = ps.tile([2, BH], f32, tag="yp")
            nc.tensor.matmul(yp[:], yt2[:], wbi[:, R2:R2 + BH],
                             start=True, stop=True)
            yrow = sb.tile([2, BH], f32, tag="yrow")
            nc.scalar.activation(yrow[:], yp[:], AF.Identity)
            nc.sync.dma_start(d_y[:], yrow[:])

    nc.finalize()
    return nc


def _fast_path_inputs(x, mask, e_low, e_high, tau_lo, tau_hi, kappa, t, head_w):
    """Build the per-core input maps (host work = transposes/slicing only)."""
    khalf = _F32(kappa) / _F32(2.0)
    blo = (khalf * tau_lo).astype(_F32).reshape(KT, 128)
    bhi = (-khalf * tau_hi).astype(_F32).reshape(KT, 128)
    xT = np.ascontiguousarray(x.T, dtype=_F32)  # (D, B)
    maskT = mask.T.reshape(KT, 128, R)
    elT = e_low.T.reshape(KT, 128, R)
    ehT = e_high.T.reshape(KT, 128, R)
    w_row = head_w.reshape(R).astype(_F32)

    xbs = []
    for i in range(NB):
        xb = np.empty((KT, 128, B2 + 2), dtype=_F32)
        xb[:, :, :B2] = xT[:, i * B2:(i + 1) * B2].reshape(KT, 128, B2)
        xb[:, :, B2] = blo
        xb[:, :, B2 + 1] = bhi
        xbs.append(xb)
    shards = []
    for j in range(NR):
        rs = slice(j * R2, (j + 1) * R2)
        wbi = np.empty((BH, R2 + BH), dtype=_F32)
        wbi[:, :R2] = w_row[rs]
        wbi[:, R2:] = np.eye(BH, dtype=_F32)
        shards.append({
            "maskT": np.ascontiguousarray(maskT[:, :, rs], dtype=_F32),
            "elT": np.ascontiguousarray(elT[:, :, rs], dtype=_F32),
            "ehT": np.ascontiguousarray(ehT[:, :, rs], dtype=_F32),
            "t_row": np.ascontiguousarray(t[rs].reshape(1, R2), dtype=_F32),
            "wbi": wbi,
        })

    in_maps = []
    for c in range(N_CORES):
        i, j = c % NB, c // NB
        in_maps.append({"xb": xbs[i], **shards[j]})
    return in_maps, float(-khalf), float(khalf)


def _reference_numpy(x, center, log_width, e_low, e_high, mask, log_kappa, t,
                     head_w, head_b):
    """General fallback, exact reference semantics in fp32 numpy (chunked)."""
    width = np.clip(np.exp(log_width, dtype=_F32), 1e-3, 50.0).astype(_F32)
    t_low = (center - _F32(0.5) * width).astype(_F32)
    t_high = (center + _F32(0.5) * width).astype(_F32)
    kappa = np.clip(np.exp(_F32(log_kappa)), 0.5, 50.0).astype(_F32)

    def sig(v):
        return _F32(0.5) * (np.tanh(_F32(0.5) * v) + _F32(1.0))

    m = sig(mask.astype(_F32))
    el = np.tanh(e_low.astype(_F32))
    eh = np.tanh(e_high.astype(_F32))
    out = np.empty(x.shape[0], dtype=_F32)
    for s in range(0, x.shape[0], 64):
        xc = x[s:s + 64].astype(_F32)
        low = sig(kappa * (t_low[None] - xc[:, None, :]))
        high = sig(kappa * (xc[:, None, :] - t_high[None]))
        evidence = np.sum(
            m[None] * (el[None] * (2 * low - 1) + eh[None] * (2 * high - 1)),
            axis=2, dtype=_F32)
        z = sig(_F32(BETA) * (evidence - t[None].astype(_F32)))
        out[s:s + 64] = z @ head_w.reshape(-1).astype(_F32) + _F32(head_b)
    return out


def kernel_with_stats(trace=False, **inputs):
    x = np.asarray(inputs["x"], dtype=_F32)
    center = np.asarray(inputs["center"], dtype=_F32)
    log_width = np.asarray(inputs["log_width"], dtype=_F32)
    e_low = np.asarray(inputs["e_low"], dtype=_F32)
    e_high = np.asarray(inputs["e_high"], dtype=_F32)
    mask = np.asarray(inputs["mask"], dtype=_F32)
    log_kappa = np.asarray(inputs["log_kappa"], dtype=_F32)
    t = np.asarray(inputs["t"], dtype=_F32)
    head_w = np.asarray(inputs["head_w"], dtype=_F32)
    head_b = np.asarray(inputs["head_b"], dtype=_F32)

    assert x.shape == (B, D) and mask.shape == (R, D)

    # fast-path structural check: thresholds constant across the rule axis
    width = np.clip(np.exp(log_width), 1e-3, 50.0).astype(_F32)
    t_low = (center - _F32(0.5) * width).astype(_F32)
    t_high = (center + _F32(0.5) * width).astype(_F32)
    if not (np.all(t_low == t_low[0:1]) and np.all(t_high == t_high[0:1])):
        out = _reference_numpy(x, center, log_width, e_low, e_high, mask,
                               log_kappa, t, head_w, head_b)
        return out, None

    from concourse.bass_utils import run_bass_kernel_spmd

    kappa = np.clip(np.exp(_F32(log_kappa)), 0.5, 50.0).astype(_F32)
    in_maps, scale_lo, scale_hi = _fast_path_inputs(
        x, mask, e_low, e_high, t_low[0], t_high[0], kappa, t, head_w)

    nc = _build_nc(scale_lo, scale_hi, float(head_b.reshape(-1)[0]) / 2.0)
    res = run_bass_kernel_spmd(nc, in_maps, list(range(N_CORES)), trace=trace)
    out = np.zeros(B, dtype=np.float64)
    for c in range(N_CORES):
        i = c % NB
        out[i * B2:(i + 1) * B2] += res.results[c]["y"].reshape(B2).astype(np.float64)
    return out.astype(_F32), res


def kernel(**inputs):
    out, _ = kernel_with_stats(**inputs)
    return out
